# revision 19
# baseline (speedup 1.0000x reference)
"""Trainium2 Bass kernel for nn_CholecMetric (segment_reduce).

Per-core (1 clip per NeuronCore, data-parallel over N=8):
  score[h,w] = (sum_p iog_max[p] * Gp[p,h,w]) / (sum_p Gp[p,h,w])
  where iog_max[p] = max_t |Gp_p & Gt_t| / |Gt_t|   (0 where undefined)

Layout: hw = k*512 + c, k on partitions. Both inputs are host-transposed
so the batched matmul operands are single-stride contiguous APs:
  gt: [K, C, T] int32 in DRAM -> SBUF bf16 [K, C*T]
  gp: [K, C, P+1] int32 (ones col appended on host) -> SBUF bf16 [K, C, PP]

Intersections via batched cross-product matmuls: group g concatenates 8
c-slices (lhsT = gt cols 128g..128g+128, rhs = gp cols 264g..) into one
[128]x[264] matmul accumulating psum[128, 264]; the 8 diagonal 16x33
blocks are the real per-slice-group results, folded with id128-selector
matmuls. Two psum halves (groups 0-31 / 32-63) so half the fold hides
under the DMA window. Cover is a contiguous innermost reduce (includes
the ones col: R = cover+1, fixed with a fused (-1, max .5) tensor_scalar).
"""

import numpy as np

import concourse.bass as bass
import concourse.bacc as bacc
import concourse.tile as tile
from concourse import mybir
from concourse.bass_utils import run_bass_kernel_spmd

N, P, T, H, W = 8, 32, 16, 256, 256
HW = H * W          # 65536
K, C = 128, 512     # hw = k*C + c
NCORES = 8
S = 8               # c-slices per batched matmul group
G = C // S          # 64 groups
PP = P + 1          # 33 rhs cols per slice (incl ones col)

F32 = mybir.dt.float32
F16 = mybir.dt.bfloat16
I32 = mybir.dt.int32
ALU = mybir.AluOpType

# phase-B split: DVE STT chain p in [0,NP_DVE); ACT mul -> GPSIMD add for
# p in [NP_DVE,NP_GPS); ACT mul -> DVE add for p in [NP_GPS,P)
NP_DVE = 14
NP_GPS = 23


def build():
    nc = bacc.Bacc("TRN2", target_bir_lowering=False, debug=False,
                   num_devices=1)
    gp_d = nc.dram_tensor("gp", [K, C * PP], I32, kind="ExternalInput")
    gt_d = nc.dram_tensor("gt", [K, C * T], I32, kind="ExternalInput")
    id16_d = nc.dram_tensor("id16", [T, T], F32, kind="ExternalInput")
    id32_d = nc.dram_tensor("id32", [P, P], F32, kind="ExternalInput")
    id128_d = nc.dram_tensor("id128", [K, K], F32, kind="ExternalInput")
    out_d = nc.dram_tensor("score", [HW], F32, kind="ExternalOutput")

    gp_r = gp_d.rearrange("k (c p) -> k c p", p=PP)  # [128, 512, 33]
    out_r = out_d.rearrange("(k c) -> k c", c=C)     # [128, 512]

    with tile.TileContext(nc) as tc:
        with (
            tc.tile_pool(name="data", bufs=1) as data,
            tc.tile_pool(name="work", bufs=1) as work,
            tc.tile_pool(name="small", bufs=1) as small,
            tc.tile_pool(name="psum", bufs=1, space="PSUM") as psum,
        ):
            gp_t = data.tile([K, C, PP], F16, tag="gp")
            gt_t = data.tile([K, C * T], F16, tag="gt")

            id16 = small.tile([T, T], F32, tag="id16")
            id32 = small.tile([P, P], F32, tag="id32")
            id128 = small.tile([K, K], F32, tag="id128")
            ones128 = small.tile([1, K], F32, tag="ones128")
            nc.sync.dma_start(out=id16[:], in_=id16_d[:])
            nc.sync.dma_start(out=id32[:], in_=id32_d[:])
            nc.sync.dma_start(out=id128[:], in_=id128_d[:])
            nc.vector.memset(ones128[:], 1.0)

            # paired c-chunk SWDGE cast loads (int32 -> bf16); contiguous
            # multi-KB lines per partition; last gp chunks small so the
            # final matmul release is short
            def ld_gt(c0, c1):
                nc.gpsimd.dma_start(out=gt_t[:, T * c0:T * c1],
                                    in_=gt_d[:, T * c0:T * c1])

            def ld_gp(c0, c1):
                nc.gpsimd.dma_start(out=gp_t[:, c0:c1, :],
                                    in_=gp_r[:, c0:c1, :])

            ld_gt(0, 128)
            ld_gp(0, 128)
            ld_gt(128, 256)
            ld_gp(128, 256)
            ld_gt(256, 384)
            ld_gp(256, 384)
            ld_gt(384, 512)
            ld_gp(384, 448)
            ld_gp(448, 512)

            # batched intersections, two psum halves; all 64 groups run
            # back-to-back on PE (no mid-stream fold: a fold that waits on
            # a vector-engine psum copy stalls the in-order PE stream).
            # psum copies go to the idle ACT engine; A-copy overlaps the
            # second half of the matmul stream.
            psum_iA = psum.tile([K, S * PP], F32, tag="intersA")
            psum_iB = psum.tile([K, S * PP], F32, tag="intersB")
            for g in range(G // 2):
                nc.tensor.matmul(psum_iA[:], gt_t[:, S * T * g:S * T * (g + 1)],
                                 gp_t[:, S * g:S * (g + 1), :],
                                 start=(g == 0), stop=(g == G // 2 - 1))
            for g in range(G // 2, G):
                nc.tensor.matmul(psum_iB[:], gt_t[:, S * T * g:S * T * (g + 1)],
                                 gp_t[:, S * g:S * (g + 1), :],
                                 start=(g == G // 2), stop=(g == G - 1))
            inters_sA = small.tile([K, S * PP], F32, tag="inters_sA")
            nc.scalar.copy(inters_sA[:], psum_iA[:])
            inters_sB = small.tile([K, S * PP], F32, tag="inters_sB")
            nc.scalar.copy(inters_sB[:], psum_iB[:])
            psum2 = psum.tile([T, PP], F32, tag="int2")
            for i in range(S):
                nc.tensor.matmul(psum2[:], id128[:, T * i:T * i + T],
                                 inters_sA[:, PP * i:PP * i + PP],
                                 start=(i == 0), stop=False,
                                 skip_group_check=True)
            for i in range(S):
                nc.tensor.matmul(psum2[:], id128[:, T * i:T * i + T],
                                 inters_sB[:, PP * i:PP * i + PP],
                                 start=False, stop=(i == S - 1),
                                 skip_group_check=True)

            # cover: contiguous innermost reduce per chunk; R = cover + 1
            # (ones col included) -> covm = max(R - 1, 0.5); rcov = 1/covm
            covm = work.tile([K, C], F32, tag="covm")
            rcov = work.tile([K, C], F32, tag="rcov")
            for c0, c1 in ((0, 128), (128, 256), (256, 384), (384, 512)):
                nc.vector.tensor_reduce(covm[:, c0:c1], gp_t[:, c0:c1, :],
                                        mybir.AxisListType.X, ALU.add)
                nc.vector.tensor_scalar(covm[:, c0:c1], covm[:, c0:c1],
                                        -1.0, 0.5, ALU.add, ALU.max)
                nc.vector.reciprocal(rcov[:, c0:c1], covm[:, c0:c1])

            # w-chain
            areag = small.tile([T, 1], F32, tag="areag")
            nc.vector.tensor_scalar_max(areag[:], psum2[:, P:PP], 0.5)
            rarea = small.tile([T, 1], F32, tag="rarea")
            nc.vector.reciprocal(rarea[:], areag[:])
            iogs = small.tile([T, P], F32, tag="iogs")
            nc.vector.tensor_scalar_mul(iogs[:], psum2[:, 0:P], rarea[:, 0:1])
            psum_tr = psum.tile([P, T], F32, tag="tr")
            nc.tensor.transpose(psum_tr[:], iogs[:], id16[:])
            iomax = small.tile([P, 1], F32, tag="iomax")
            nc.vector.tensor_reduce(iomax[:], psum_tr[:],
                                    mybir.AxisListType.X, ALU.max)
            psum_wr = psum.tile([1, P], F32, tag="wr")
            nc.tensor.matmul(psum_wr[:], iomax[:], id32[:])
            w_row = small.tile([1, P], F32, tag="wrow")
            nc.vector.tensor_copy(w_row[:], psum_wr[:])
            psum_wb = psum.tile([K, P], F32, tag="wb")
            nc.tensor.matmul(psum_wb[:], ones128[:], w_row[:])
            w_bc = small.tile([K, P], F32, tag="wbc")
            nc.vector.tensor_copy(w_bc[:], psum_wb[:])

            # num = sum_p w[p] * Gp[p]: three chains on strided p-slices
            def gp_p(p):
                return gp_t[:, :, p]  # [128, 512] stride PP

            accv = work.tile([K, C], F16, tag="accv")
            accg = work.tile([K, C], F16, tag="accg")
            nc.vector.tensor_scalar_mul(accv[:], gp_p(0), w_bc[:, 0:1])
            for p in range(1, NP_DVE):
                nc.vector.scalar_tensor_tensor(
                    accv[:], gp_p(p), w_bc[:, p:p + 1], accv[:],
                    ALU.mult, ALU.add)
            nc.scalar.mul(accg[:], gp_p(NP_DVE), w_bc[:, NP_DVE:NP_DVE + 1])
            for p in range(NP_DVE + 1, NP_GPS):
                tmp = work.tile([K, C], F16, tag=f"tmpg{p}")
                nc.scalar.mul(tmp[:], gp_p(p), w_bc[:, p:p + 1])
                nc.gpsimd.tensor_tensor(accg[:], accg[:], tmp[:], ALU.add)
            for p in range(NP_GPS, P):
                tmp = work.tile([K, C], F16, tag=f"tmpv{p}")
                nc.scalar.mul(tmp[:], gp_p(p), w_bc[:, p:p + 1])
                nc.vector.tensor_tensor(accv[:], accv[:], tmp[:], ALU.add)

            nc.vector.tensor_tensor(accv[:], accv[:], accg[:], ALU.add)
            score = work.tile([K, C], F32, tag="score")
            nc.vector.tensor_tensor(score[:], accv[:], rcov[:], ALU.mult)

            nc.sync.dma_start(out=out_r[:], in_=score[:])

    nc.compile()
    return nc


_NC_CACHE = None


def _get_nc():
    global _NC_CACHE
    if _NC_CACHE is None:
        _NC_CACHE = build()
    return _NC_CACHE


def kernel(groups_pred: np.ndarray, groups_true: np.ndarray, trace=False,
           **trace_kwargs) -> np.ndarray:
    nc = _get_nc()
    gp = np.asarray(groups_pred, dtype=np.int32).reshape(N, P, K, C)
    gt = np.asarray(groups_true, dtype=np.int32).reshape(N, T, K, C)
    # host-side layout prep (not on the device critical path):
    # gp -> [K, C, P+1] with a ones column appended; gt -> [K, C, T]
    gp4 = np.empty((N, K, C, PP), dtype=np.int32)
    gp4[..., :P] = gp.transpose(0, 2, 3, 1)
    gp4[..., P] = 1
    gpT = np.ascontiguousarray(gp4).reshape(N, K, C * PP)
    gtT = np.ascontiguousarray(gt.transpose(0, 2, 3, 1)).reshape(N, K, C * T)
    id16 = np.eye(T, dtype=np.float32)
    id32 = np.eye(P, dtype=np.float32)
    id128 = np.eye(K, dtype=np.float32)
    in_maps = [{"gp": gpT[n], "gt": gtT[n], "id16": id16, "id32": id32,
                "id128": id128} for n in range(N)]
    res = run_bass_kernel_spmd(nc, in_maps, list(range(NCORES)), trace=trace,
                               **trace_kwargs)
    out = np.stack([res.results[n]["score"].reshape(H, W) for n in range(N)])
    if trace:
        kernel.last_results = res
    return out.astype(np.float32)
